# revision 1
# baseline (speedup 1.0000x reference)
"""Multi-head self-attention (B=4, S=2048, E=1024, H=16, D=64) on 8 TRN2 cores.

Sharding: core c handles batch b = c//2 and head-group hg = c%2 (8 of 16 heads).
QKV weights column-parallel, proj row-parallel (Megatron); the two cores
sharing a batch produce partial proj outputs that are summed on the host.

Device layout (per core):
  xt  = x[b].T                    [E=1024, S=2048]  (host pre-transposed)
  qT/kT = (x Wq)^T etc            [512 feats, 2048] feats on partitions
  scores computed transposed:     S_T[k, q] = K^T.T @ Q^T  (pair-packed on
                                  the 64-wide contraction via base_partition)
  softmax without max-subtraction (scores are ~N(0,1); exp cannot overflow),
  denominator fused into the O matmul via a ones-augmented [V | 1] stationary.
  O^T[d, q] accumulated over k-chunks; normalized by PE-broadcast reciprocal.
  proj: out[s, n] = attnT.T @ Wp slice, accumulated over feature chunks.

All matmuls run in float32r (TF32-like, full PE rate; rel err ~1e-4).
"""

import numpy as np

B, S, E = 4, 2048, 1024
H, D = 16, 64
HLOC = 8          # heads per core
FEAT = HLOC * D   # 512 per-core q/k/v features
NCORES = 8

_CACHE = {}


def _build_program(has_bqkv, has_bp):
    import concourse.bass as bass
    import concourse.mybir as mybir
    from concourse import bacc
    from concourse.tile import TileContext

    F32R = mybir.dt.float32r
    F32 = mybir.dt.float32
    F16 = mybir.dt.float16
    AF = mybir.ActivationFunctionType

    nc = bacc.Bacc("TRN2", target_bir_lowering=False, num_devices=NCORES)

    xt = nc.dram_tensor("xt", [E, S], F16, kind="ExternalInput")
    wq = nc.dram_tensor("wq", [E, FEAT], F16, kind="ExternalInput")
    wk = nc.dram_tensor("wk", [E, FEAT], F16, kind="ExternalInput")
    wv = nc.dram_tensor("wv", [E, FEAT], F16, kind="ExternalInput")
    wp = nc.dram_tensor("wp", [FEAT, E], F16, kind="ExternalInput")
    out = nc.dram_tensor("out", [S, E], F32, kind="ExternalOutput")
    if has_bqkv:
        bq_d = nc.dram_tensor("bq", [128, 4], F32, kind="ExternalInput")
        bk_d = nc.dram_tensor("bk", [128, 4], F32, kind="ExternalInput")
        bv_d = nc.dram_tensor("bv", [1, FEAT], F32R, kind="ExternalInput")
    if has_bp:
        bp_d = nc.dram_tensor("bp", [1, E], F32R, kind="ExternalInput")

    EC = E // 128      # 8 e-chunks (contraction for qkv)
    SC = S // 512      # 4 s-chunks of 512
    KC = S // 128      # 16 k-chunks of 128
    NP = HLOC // 2     # 4 head pairs

    xt_v = xt.ap().rearrange("(c p) s -> p c s", p=128)
    wq_v = wq.ap().rearrange("(c p) f -> p c f", p=128)
    wk_v = wk.ap().rearrange("(c p) f -> p c f", p=128)
    wv_v = wv.ap().rearrange("(c p) f -> p c f", p=128)
    wp_v = wp.ap().rearrange("(c p) n -> p c n", p=128)

    CH = 512           # x-stream chunk width (s)
    NCH = S // CH      # 4 chunks

    with TileContext(nc) as tc:
      from contextlib import ExitStack
      from collections import deque
      with ExitStack() as es:
        pp = es.enter_context(tc.tile_pool(name="persist", bufs=1))
        pqk = es.enter_context(tc.tile_pool(name="pqk", bufs=1))

        qT = pqk.tile([128, NP, S], F16)     # [feat128, pair, s]
        kT = pqk.tile([128, NP, S], F16)
        v1 = pqk.tile([128, KC, HLOC, D + 1], F16)  # [k128, kc, head, V|1]
        ones1 = pp.tile([1, 128], F32R)
        neg4 = pp.tile([128, 1], F32)
        nc.gpsimd.memset(neg4[:], -4.0)

        px = es.enter_context(tc.tile_pool(name="px", bufs=2))
        pwqk = es.enter_context(tc.tile_pool(name="pwqk", bufs=1))
        pwp = es.enter_context(tc.tile_pool(name="pwp", bufs=1))
        pout = es.enter_context(tc.tile_pool(name="pout", bufs=3))
        wq_sb = pwqk.tile([128, EC, FEAT], F16)
        wk_sb = pwqk.tile([128, EC, FEAT], F16)
        wp_sb = pwp.tile([128, NP, E], F16)
        nc.sync.dma_start(wk_sb[:, 0:EC // 2], wk_v[:, 0:EC // 2])
        nc.sync.dma_start(wk_sb[:, EC // 2:], wk_v[:, EC // 2:])
        nc.sync.dma_start(wq_sb[:], wq_v)

        # constants: ones row + the ones column of [V | 1] (ACT const fill)
        nc.scalar.activation(ones1[:], wk_sb[0:1, 0, 0:128],
                             AF.Copy, bias=1.0, scale=0.0)
        nc.scalar.activation(
            v1[:, :, :, D],
            wk_sb[:, 0, 0:KC * HLOC].rearrange("p (a b) -> p a b", a=KC),
            AF.Copy, bias=1.0, scale=0.0)

        if has_bqkv:
            bq_sb = pp.tile([128, 4], F32)
            bk_sb = pp.tile([128, 4], F32)
            bv_row = pp.tile([1, FEAT], F32R)
            nc.sync.dma_start(bq_sb[:], bq_d[:])
            nc.sync.dma_start(bk_sb[:], bk_d[:])
            nc.sync.dma_start(bv_row[:], bv_d[:])
            bv_bc = pp.tile([128, FEAT], F32)

        def qkv_copy(dst_ap, ps_ap, which, fc):
            """psum -> sbuf (fp16) with optional per-partition bias."""
            with nc.allow_low_precision(reason="fp16 attn"):
                if has_bqkv and which in ("q", "k"):
                    bias_ap = (bq_sb if which == "q" else bk_sb)[:, fc]
                    nc.scalar.activation(dst_ap, ps_ap, AF.Copy, bias=bias_ap)
                else:
                    nc.vector.tensor_copy(dst_ap, ps_ap)

        # ---------- Phase B pools (opened early: block (0,0) is interleaved
        # with pass 1) ----------
        pAT = es.enter_context(tc.tile_pool(name="pAT", bufs=1))
        attnT = pAT.tile([128, NP, S], F16)
        with tc.tile_pool(name="pwv", bufs=1) as pwv, \
             tc.tile_pool(name="pB", bufs=1) as pB, \
             tc.tile_pool(name="psB", bufs=3, space="PSUM") as psB, \
             tc.tile_pool(name="psO", bufs=1, space="PSUM") as psO:

            wv_sb = pwv.tile([128, EC, FEAT], F16)
            nc.sync.dma_start(wv_sb[:], wv_v)
            if has_bqkv:
                ps_bv = psB.tile([128, FEAT], F32, tag="ps_s")
                nc.tensor.matmul(ps_bv[:], ones1[:], bv_row[:],
                                 start=True, stop=True)
                nc.vector.tensor_copy(bv_bc[:], ps_bv[:])
            if has_bp:
                bp_row = pB.tile([1, E], F32R, tag="bp_row")
                nc.sync.dma_start(bp_row[:], bp_d[:])
                bp_bc = pB.tile([128, E], F32, tag="bp_bc")
                ps_bp = psB.tile([128, 1024], F32, tag="ps_s", name="ps_bp")
                nc.tensor.matmul(ps_bp[:, 0:512], ones1[:], bp_row[:, 0:512],
                                 start=True, stop=True)
                nc.tensor.matmul(ps_bp[:, 512:], ones1[:], bp_row[:, 512:],
                                 start=True, stop=True)
                nc.vector.tensor_copy(bp_bc[:], ps_bp[:])

            def qk_unit(dst, w_sb, which, fc, ch):
                """Q/K projection for one (pair, x-chunk): 8 matmul steps."""
                xt_t = px.tile([128, EC, CH], F16, tag="xt", name="xt_t")
                nc.sync.dma_start(xt_t[:], xt_v[:, :, ch * CH:(ch + 1) * CH])
                ps1 = psB.tile([128, CH], F32, tag="ps_s", name="ps1")
                for ec in range(EC):
                    nc.tensor.matmul(
                        ps1[:], w_sb[:, ec, fc * 128:(fc + 1) * 128],
                        xt_t[:, ec, :],
                        start=(ec == 0), stop=(ec == EC - 1))
                    yield
                qkv_copy(dst[:, fc, slice(ch * CH, (ch + 1) * CH)],
                         ps1[:], which, fc)

            def proj_unit(sc):
                """out[sc*128:+128, :] = attnT.T @ wp: 8 matmul steps."""
                ssl = slice(sc * 128, (sc + 1) * 128)
                ps_p = psB.tile([128, 1024], F32, tag="ps_s", name="ps_p")
                for n2 in range(2):
                    nsl = slice(n2 * 512, (n2 + 1) * 512)
                    for fc in range(NP):
                        nc.tensor.matmul(ps_p[:, nsl], attnT[:, fc, ssl],
                                         wp_sb[:, fc, nsl],
                                         start=(fc == 0), stop=(fc == NP - 1))
                        yield
                out_t = pout.tile([128, E], F32, tag="out", name="out_t")
                with nc.allow_low_precision(reason="fp16 attn"):
                    if has_bp:
                        nc.vector.tensor_add(out_t[:], ps_p[:], bp_bc[:])
                    else:
                        nc.vector.tensor_copy(out_t[:], ps_p[:])
                nc.sync.dma_start(out.ap()[ssl, :], out_t[:])

            work = deque()
            qk_steps = [0]
            for fc in range(1, NP):
                for ch in range(NCH):
                    work.append(qk_unit(kT, wk_sb, "k", fc, ch))
                    qk_steps[0] += EC
                for ch in range(NCH):
                    work.append(qk_unit(qT, wq_sb, "q", fc, ch))
                    qk_steps[0] += EC

            def pull(n):
                while n > 0 and work:
                    try:
                        next(work[0])
                        n -= 1
                        qk_steps[0] -= 1
                    except StopIteration:
                        work.popleft()

            def make_norm(p, qc, ps_oA, ps_oB):
                def norm():
                    qsl = slice(qc * 512, (qc + 1) * 512)
                    den = pB.tile([1, 1024], F32R, tag="den", bufs=2,
                                  name="den")
                    with nc.allow_low_precision(reason="fp16 attn"):
                        nc.vector.tensor_copy(den[:, 0:512], ps_oA[D:D + 1, :])
                        nc.vector.tensor_copy(den[:, 512:], ps_oB[D:D + 1, :])
                    ps_b = psB.tile([64, 1024], F32, tag="ps_s", name="ps_b")
                    nc.tensor.matmul(ps_b[:, 0:512], ones1[:, 0:64],
                                     den[:, 0:512], start=True, stop=True)
                    nc.tensor.matmul(ps_b[:, 512:], ones1[:, 0:64],
                                     den[:, 512:], start=True, stop=True)
                    r_sb = pB.tile([64, 1024], F32, tag="r_sb", bufs=2,
                                   name="r_sb")
                    nc.vector.reciprocal_approx_fast(out=r_sb[:], in_=ps_b[:])
                    with nc.allow_low_precision(reason="fp16 attn"):
                        nc.vector.tensor_mul(attnT[0:64, p, qsl],
                                             ps_oA[0:D, :], r_sb[:, 0:512])
                        nc.vector.tensor_mul(attnT[64:128, p, qsl],
                                             ps_oB[0:D, :], r_sb[:, 512:])
                return norm

            def attn_kc(p, qc, kc, ps_oA, ps_oB):
                qsl = slice(qc * 512, (qc + 1) * 512)
                ks = slice(kc * 128, (kc + 1) * 128)
                ps_s = psB.tile([128, 1024], F32, tag="ps_s", name="ps_s")
                nc.tensor.matmul(ps_s[:, 0:512],
                                 kT[0:64, p, ks], qT[0:64, p, qsl],
                                 start=True, stop=True)
                nc.tensor.matmul(ps_s[:, 512:],
                                 kT[64:128, p, ks], qT[64:128, p, qsl],
                                 start=True, stop=True)
                pT = pB.tile([128, 1024], F16, tag="pT", bufs=10, name="pT")
                # bias -4: softmax is shift-invariant; keeps exp well inside
                # fp16 range (raw scores reach ~8.3, exp(8.3)=4e3 vs max 6.5e4)
                with nc.allow_low_precision(reason="fp16 attn"):
                    nc.scalar.activation(pT[:], ps_s[:], AF.Exp, scale=0.125,
                                         bias=neg4[:])
                nc.tensor.matmul(ps_oA[:], v1[:, kc, 2 * p, :], pT[:, 0:512],
                                 start=(kc == 0), stop=(kc == KC - 1))
                nc.tensor.matmul(ps_oB[:], v1[:, kc, 2 * p + 1, :],
                                 pT[:, 512:],
                                 start=(kc == 0), stop=(kc == KC - 1))

            # ---- Pass 1 with interleaved block (0,0) quarters ----
            # Each x-chunk ch produces kT/qT[pair 0] s-cols and v1 k-chunks
            # for exactly kc in [4ch, 4ch+4), so block (0,0) advances right
            # behind the chunk loop and ACT starts ~60us earlier.
            ps_oA00 = psO.tile([D + 1, 512], F32, tag="oA", name="ps_oA00")
            ps_oB00 = psO.tile([D + 1, 512], F32, tag="oB", name="ps_oB00")
            if True:
                for ch in range(NCH):
                    xt_t = px.tile([128, EC, CH], F16, tag="xt")
                    csl2 = slice(ch * CH, (ch + 1) * CH)
                    nc.sync.dma_start(xt_t[:, 0:EC // 2],
                                      xt_v[:, 0:EC // 2, csl2])
                    nc.sync.dma_start(xt_t[:, EC // 2:],
                                      xt_v[:, EC // 2:, csl2])
                    csl = slice(ch * CH, (ch + 1) * CH)
                    for dst, w_sb, which in ((kT, wk_sb, "k"), (qT, wq_sb, "q")):
                        ps1 = psB.tile([128, CH], F32, tag="ps_s", name="ps1")
                        for ec in range(EC):
                            nc.tensor.matmul(
                                ps1[:], w_sb[:, ec, 0:128], xt_t[:, ec, :],
                                start=(ec == 0), stop=(ec == EC - 1))
                        qkv_copy(dst[:, 0, csl], ps1[:], which, 0)
                    for sc2 in range(CH // 128):
                        kcg = (ch * CH) // 128 + sc2
                        ps1 = psB.tile([128, FEAT], F32, tag="ps_s", name="ps1")
                        for ec in range(EC):
                            nc.tensor.matmul(
                                ps1[:], xt_t[:, ec, sc2 * 128:(sc2 + 1) * 128],
                                wv_sb[:, ec, :],
                                start=(ec == 0), stop=(ec == EC - 1))
                        ps1v = ps1.rearrange("p (h d) -> p h d", h=HLOC)
                        with nc.allow_low_precision(reason="fp16 attn"):
                            if has_bqkv:
                                bvv = bv_bc.rearrange("p (h d) -> p h d",
                                                      h=HLOC)
                                nc.vector.tensor_add(v1[:, kcg, :, 0:D],
                                                     ps1v, bvv)
                            else:
                                nc.vector.tensor_copy(v1[:, kcg, :, 0:D],
                                                      ps1v)
                    for kc in range(4 * ch, 4 * ch + 4):
                        attn_kc(0, 0, kc, ps_oA00, ps_oB00)

            nc.sync.dma_start(wp_sb[:], wp_v)

            # ---- Remaining blocks ----
            pending_norm = make_norm(0, 0, ps_oA00, ps_oB00)
            blocks = [(p, qc) for p in range(NP) for qc in range(SC)][1:]
            qk_blocks_left = sum(1 for (p, _) in blocks if p < 3)
            for p, qc in blocks:
                ps_oA = psO.tile([D + 1, 512], F32, tag="oA", name="ps_oA")
                ps_oB = psO.tile([D + 1, 512], F32, tag="oB", name="ps_oB")
                if p < 3:
                    rate = -(-qk_steps[0] // (16 * qk_blocks_left))
                    qk_blocks_left -= 1
                else:
                    rate = 2
                for kc in range(KC):
                    attn_kc(p, qc, kc, ps_oA, ps_oB)
                    if kc == 2 and pending_norm is not None:
                        pending_norm()
                        pending_norm = None
                    pull(rate)
                if p == 3:
                    make_norm(p, qc, ps_oA, ps_oB)()
                    for sc in range(qc * 4, qc * 4 + 4):
                        work.append(proj_unit(sc))
                else:
                    pending_norm = make_norm(p, qc, ps_oA, ps_oB)
            pull(10 ** 9)   # drain remaining proj units

    nc.compile()
    return nc


def _prep_inputs(x, W_qkv, b_qkv, W_proj, b_proj, has_bqkv, has_bp):
    """Build the 8 per-core input maps (host-side sharding/layout only)."""
    Wr = np.ascontiguousarray(W_qkv.reshape(E, 3, H, D))
    in_maps = []
    for c in range(NCORES):
        b, hg = c // 2, c % 2
        hsl = slice(hg * HLOC, (hg + 1) * HLOC)
        m = {
            "xt": np.ascontiguousarray(x[b].T).astype(np.float16),
            "wq": np.ascontiguousarray(Wr[:, 0, hsl, :].reshape(E, FEAT)).astype(np.float16),
            "wk": np.ascontiguousarray(Wr[:, 1, hsl, :].reshape(E, FEAT)).astype(np.float16),
            "wv": np.ascontiguousarray(Wr[:, 2, hsl, :].reshape(E, FEAT)).astype(np.float16),
            "wp": np.ascontiguousarray(
                W_proj[hg * FEAT:(hg + 1) * FEAT, :]).astype(np.float16),
        }
        if has_bqkv:
            br = b_qkv.reshape(3, H, D)
            m["bq"] = np.ascontiguousarray(
                br[0, hsl, :].reshape(4, 128).T)
            m["bk"] = np.ascontiguousarray(
                br[1, hsl, :].reshape(4, 128).T)
            m["bv"] = np.ascontiguousarray(br[2, hsl, :].reshape(1, FEAT))
        if has_bp:
            m["bp"] = np.ascontiguousarray((b_proj * 0.5).reshape(1, E))
        in_maps.append(m)
    return in_maps


def run(x, W_qkv, b_qkv, W_proj, b_proj, trace=False):
    from concourse.bass_utils import run_bass_kernel_spmd

    has_bqkv = bool(np.any(b_qkv))
    has_bp = bool(np.any(b_proj))
    key = (has_bqkv, has_bp)
    if key not in _CACHE:
        _CACHE[key] = _build_program(has_bqkv, has_bp)
    nc = _CACHE[key]

    in_maps = _prep_inputs(x, W_qkv, b_qkv, W_proj, b_proj, has_bqkv, has_bp)
    res = run_bass_kernel_spmd(nc, in_maps, core_ids=list(range(NCORES)),
                               trace=trace)
    out = np.empty((B, S, E), dtype=np.float32)
    for b in range(B):
        out[b] = res.results[2 * b]["out"] + res.results[2 * b + 1]["out"]
    return out, res


def kernel(x, W_qkv, b_qkv, W_proj, b_proj):
    out, _ = run(np.asarray(x), np.asarray(W_qkv), np.asarray(b_qkv),
                 np.asarray(W_proj), np.asarray(b_proj))
    return out

